# revision 23
# baseline (speedup 1.0000x reference)
"""Causal multi-head attention (B=2, S=2048, D=1024, H=16) on 8 NeuronCores.

Sharding: batch x head-group parallel. Core c owns batch b = c//4 and head
group g = c%4 (heads 4g..4g+3 = projection dims 256g..256g+256). Each core
computes a [2048, 1024] partial of the final projection for its batch; the
host sums the 4 partials per batch (row-parallel wo over head groups).

vs the fully-replicated head-parallel layout this halves both the x read and
the partial-output write per core.

All matmul operands are fp16 (fp32 PSUM accumulation): fp16 streams at full
PE rate with no small-N penalty, halves DMA bytes, and keeps ~1e-3 rel err.

Kernel layout: scores are computed transposed (scoresT[k, q]) per head-pair
so softmax probs come out k-partitioned and feed attn@v directly. The v
projection is computed transposed (x-tile stationary) so v lands [kpos, dim]
with no transpose step. A ones-column at each end of the v tile (col 0 for
the even head, col 129 for the odd head) makes the same attn@v matmul emit
the softmax denominators: stationary slices 0:65 and 65:130 are contiguous.
Scores are small (|s/8| < ~3) so softmax without max-subtraction is exact.

Pipeline: projection s-chunk qc+1 is emitted before attention q-chunk qc so
the PE fills ACT-bound softmax stretches with projection matmuls.
"""
import numpy as np

import concourse.bass as bass
import concourse.tile as tile
from concourse import bacc, mybir
from concourse.bass_utils import run_bass_kernel_spmd

B, S, D = 2, 2048, 1024
H, HD = 16, 64
NCORES = 8
CH = 512                # q/s chunk width
KT = 128                # k-tile (keys per tile)
NCH = S // CH           # 4 chunks per core
NEG = -60000.0          # causal mask additive constant (finite in fp32)

F32 = mybir.dt.float32
F16 = mybir.dt.float16

_cache = {}


def _emit_body(nc, tc, io, rep):
    xt, wqt, wkt, wvt, wot, maskt, outp = io
    Exp = mybir.ActivationFunctionType.Exp
    r_ = f"r{rep}_"

    with tc.tile_pool(name=r_ + "persist", bufs=1) as persist, \
         tc.tile_pool(name=r_ + "pj_ps", bufs=2, space="PSUM") as pj_ps, \
         tc.tile_pool(name=r_ + "sc_ps", bufs=2, space="PSUM") as sc_ps, \
         tc.tile_pool(name=r_ + "out_ps", bufs=2, space="PSUM") as out_ps, \
         tc.tile_pool(name=r_ + "et_p", bufs=6) as et_p, \
         tc.tile_pool(name=r_ + "sums_p", bufs=4) as sums_p, \
         tc.tile_pool(name=r_ + "stg_p", bufs=4) as stg_p:

        x_s = persist.tile([128, 8, S], F16)       # [xdim%128, xdim//128, s]
        qT = persist.tile([128, 2, S], F16)        # [pair-dim, pair, s]
        kT = persist.tile([128, 2, S], F16)
        vN = persist.tile([128, 16, 2, 256], F16)  # [s%128, st, pair, d|1s|d'|1s]
        oT = persist.tile([128, 2, S], F16)        # normalized attn out, T
        wq_s = persist.tile([128, 8, 2, 128], F16)
        wk_s = persist.tile([128, 8, 2, 128], F16)
        wv_s = persist.tile([128, 8, 256], F16)
        wo_s = persist.tile([128, 2, D], F16)
        mk_s = persist.tile([128, 2, 128], F16)    # multiplicative 0/1 mask

        xr = xt.ap().rearrange("t p s -> p t s")
        nc.sync.dma_start(wv_s[:], wvt.ap().rearrange("t p j -> p t j"))
        nc.sync.dma_start(x_s[:, :, 0:128], xr[:, :, 0:128])
        nc.sync.dma_start(x_s[:, :, 128:CH], xr[:, :, 128:CH])
        nc.sync.dma_start(wq_s[:], wqt.ap().rearrange("t p dt m -> p t dt m"))
        nc.sync.dma_start(wk_s[:], wkt.ap().rearrange("t p dt m -> p t dt m"))
        for c in range(1, 4):
            col = slice(c * CH, (c + 1) * CH)
            nc.sync.dma_start(x_s[:, :, col], xr[:, :, col])
        nc.sync.dma_start(wo_s[:], wot.ap().rearrange("k p o -> p k o"))
        nc.sync.dma_start(mk_s[:], maskt.ap())
        # softmax-denominator ones blocks: per head 64 ones-columns so the
        # attn@v matmul replicates the denominator across 64 psum rows
        nc.vector.memset(vN[:, :, :, 64:128], 1.0)
        nc.vector.memset(vN[:, :, :, 192:256], 1.0)

        def proj_groups(c):
            """Emitters projecting s-chunk c (512 rows) into qT/kT/vN."""
            col = slice(c * CH, (c + 1) * CH)

            def qk(dt, w_s, dst, nm):
                ps = pj_ps.tile([128, CH], F32, tag="pj",
                                name=f"ps{nm}_{rep}_{c}_{dt}")
                for t in range(8):
                    nc.tensor.matmul(ps[:], w_s[:, t, dt, :], x_s[:, t, col],
                                     start=(t == 0), stop=(t == 7))
                nc.vector.tensor_copy(dst[:, dt, col], ps[:])

            def vproj(j):
                st = 4 * c + j
                psv = pj_ps.tile([128, 256], F32, tag="pj",
                                 name=f"psv_{rep}_{c}_{j}")
                for t in range(8):
                    nc.tensor.matmul(psv[:],
                                     x_s[:, t, st * 128:(st + 1) * 128],
                                     wv_s[:, t, :],
                                     start=(t == 0), stop=(t == 7))
                nc.vector.tensor_copy(
                    vN[:, st, :, :].rearrange(
                        "p dt (hp m) -> p dt hp m", hp=2)[:, :, :, 0:64],
                    psv[:].rearrange("p (dt hp m) -> p dt hp m", dt=2, hp=2))

            gs = []
            for j in range(4):
                gs.append(lambda j=j: vproj(j))
            for dt in range(2):
                gs.append(lambda dt=dt: qk(dt, wq_s, qT, "q"))
                gs.append(lambda dt=dt: qk(dt, wk_s, kT, "k"))
            return gs

        def wo_groups(qc):
            """Emitters for final projection of q-chunk qc (normalized oT)."""
            def wos(st4):
                soff = qc * CH + st4 * 128
                stg = stg_p.tile([128, D], F16, tag="stg",
                                 name=f"stg_{rep}_{qc}_{st4}")
                for chn in range(2):
                    psf = pj_ps.tile([128, CH], F32, tag="pj",
                                     name=f"psf_{rep}_{qc}_{st4}_{chn}")
                    for pair in range(2):
                        nc.tensor.matmul(psf[:],
                                         oT[:, pair, soff:soff + 128],
                                         wo_s[:, pair, chn * CH:(chn + 1) * CH],
                                         start=(pair == 0), stop=(pair == 1))
                    dst = stg[:, chn * CH:(chn + 1) * CH]
                    if chn == 0:
                        nc.vector.tensor_copy(dst, psf[:])
                    else:
                        nc.scalar.copy(dst, psf[:])
                    nc.sync.dma_start(
                        outp.ap()[soff:soff + 128, chn * CH:(chn + 1) * CH],
                        dst)
            return [lambda st4=st4: wos(st4) for st4 in range(4)]

        def attn_chunk(qc, pair, fillers, fine_norm=False):
            """Attention + normalize for q-chunk qc, head-pair pair.

            Pops one PE filler (proj/wo group) from `fillers` every few
            k-tiles so the PE queue never head-of-line blocks on the
            scores->exp->attn@v chain."""
            bq = slice(qc * CH, (qc + 1) * CH)
            nkt = 4 * (qc + 1)
            stride = max(1, (2 * nkt) // (len(fillers) + 1)) if fillers else 1
            ps_oA = out_ps.tile([128, CH], F32, tag="ps_o",
                                name=f"ps_oA_{rep}_{qc}_{pair}")
            ps_oB = out_ps.tile([128, CH], F32, tag="ps_o",
                                name=f"ps_oB_{rep}_{qc}_{pair}")
            pending = []  # (kt, r0, et) awaiting attn@v emission

            def emit_av(last=False):
                kt, r0, et = pending.pop(0)
                for hp, ps, vsl in ((0, ps_oA, slice(0, 128)),
                                    (1, ps_oB, slice(128, 256))):
                    nc.tensor.matmul(ps[:, r0:CH], vN[:, kt, pair, vsl],
                                     et[:, hp, r0:CH],
                                     start=(kt == 0), stop=(kt == nkt - 1),
                                     skip_group_check=True)

            for kt in range(nkt):
                r = kt * KT - qc * CH
                r0 = max(r, 0)
                ps_m = sc_ps.tile([128, 2, CH], F32, tag="ps_s",
                                  name=f"ps_m_{rep}_{qc}_{pair}_{kt}")
                for hp in range(2):
                    hsl = slice(hp * 64, hp * 64 + 64)
                    nc.tensor.matmul(
                        ps_m[:, hp, r0:CH],
                        kT[hsl, pair, kt * KT:(kt + 1) * KT],
                        qT[hsl, pair, qc * CH + r0:(qc + 1) * CH],
                        start=True, stop=True)
                et = et_p.tile([128, 2, CH], F16, tag="et",
                               name=f"et_{rep}_{qc}_{pair}_{kt}")
                nc.scalar.activation(et[:, :, r0:CH], ps_m[:, :, r0:CH],
                                     Exp, scale=0.125)
                if r >= 0:
                    # zero the upper triangle of the diagonal tile post-exp
                    nc.vector.tensor_mul(et[:, :, r:r + 128],
                                         et[:, :, r:r + 128], mk_s[:])
                pending.append((kt, r0, et))
                if len(pending) > 2:
                    emit_av()
                if fillers and kt % stride == stride - 1:
                    fillers.pop(0)()
            while pending:
                emit_av()
            # normalize: ps rows 0:64 = dims, rows 64:128 = denominator
            # replicated across 64 partitions (ones block in the stationary).
            # fine=True splits columns so wo can start on the first part.
            nh = 2 if fine_norm else 1
            hw_ = CH // nh
            for h in range(nh):
                csl = slice(h * hw_, (h + 1) * hw_)
                for hp, ps in ((0, ps_oA), (1, ps_oB)):
                    rden = sums_p.tile([64, hw_], F32, tag="rden",
                                       name=f"rden_{rep}_{qc}_{pair}_{hp}_{h}")
                    nc.vector.reciprocal(rden[:], ps[64:128, csl])
                    nc.vector.tensor_mul(
                        oT[hp * 64:hp * 64 + 64, pair,
                           qc * CH + h * hw_:qc * CH + (h + 1) * hw_],
                        ps[0:64, csl], rden[:])

        for g in proj_groups(0):
            g()
        for qc in range(NCH):
            fillers = []
            if qc + 1 < NCH:
                fillers += proj_groups(qc + 1)
            if qc > 0:
                fillers += wo_groups(qc - 1)
            attn_chunk(qc, 0, fillers)
            attn_chunk(qc, 1, fillers, fine_norm=(qc == NCH - 1))
            while fillers:
                fillers.pop(0)()
        for g in wo_groups(NCH - 1):
            g()


def _build(repeats=1):
    nc = bacc.Bacc("TRN2", target_bir_lowering=False, debug=False)
    xt = nc.dram_tensor("xt", [8, 128, S], F16, kind="ExternalInput")
    wqt = nc.dram_tensor("wqt", [8, 128, 2, 128], F16, kind="ExternalInput")
    wkt = nc.dram_tensor("wkt", [8, 128, 2, 128], F16, kind="ExternalInput")
    wvt = nc.dram_tensor("wvt", [8, 128, 256], F16, kind="ExternalInput")
    wot = nc.dram_tensor("wot", [2, 128, D], F16, kind="ExternalInput")
    maskt = nc.dram_tensor("maskt", [128, 2, 128], F16, kind="ExternalInput")
    outp = nc.dram_tensor("outp", [S, D], F16, kind="ExternalOutput")
    io = (xt, wqt, wkt, wvt, wot, maskt, outp)

    with tile.TileContext(nc) as tc:
        for rep in range(repeats):
            _emit_body(nc, tc, io, rep)
    nc.compile()
    return nc


def _mask_tile() -> np.ndarray:
    kp = np.arange(128)[:, None]
    c = np.arange(128)[None, :]
    mul = (kp <= c).astype(np.float16)
    return np.broadcast_to(mul[:, None, :], (128, 2, 128)).copy()


def make_in_maps(x, wq, wk, wv, wo):
    mask = _mask_tile()
    in_maps = []
    for c in range(NCORES):
        b, g = divmod(c, 4)
        rows = slice(g * 256, (g + 1) * 256)
        xt = np.ascontiguousarray(
            x[b].T.reshape(8, 128, S)).astype(np.float16)
        wqt = np.ascontiguousarray(
            wq[rows, :].T.reshape(8, 128, 2, 128)).astype(np.float16)
        wkt = np.ascontiguousarray(
            wk[rows, :].T.reshape(8, 128, 2, 128)).astype(np.float16)
        wvt = np.ascontiguousarray(
            wv[rows, :].T.reshape(8, 128, 256)).astype(np.float16)
        wot = np.ascontiguousarray(
            wo[:, rows].T.reshape(2, 128, D)).astype(np.float16)
        in_maps.append({
            "xt": xt, "wqt": wqt, "wkt": wkt, "wvt": wvt, "wot": wot,
            "maskt": mask,
        })
    return in_maps


def _make_runner(nc):
    """Cached jitted PJRT runner; all inputs sharded per core along axis 0."""
    import jax
    from jax.sharding import Mesh, PartitionSpec, NamedSharding
    try:
        from jax.experimental.shard_map import shard_map
    except ImportError:
        shard_map = jax.shard_map
    from concourse.bass2jax import (_bass_exec_p, install_neuronx_cc_hook,
                                    partition_id_tensor)

    install_neuronx_cc_hook()
    pname = nc.partition_id_tensor.name if nc.partition_id_tensor else None
    in_names, out_names, out_avals, zero_shapes = [], [], [], []
    for alloc in nc.m.functions[0].allocations:
        if not isinstance(alloc, mybir.MemoryLocationSet):
            continue
        name = alloc.memorylocations[0].name
        if alloc.kind == "ExternalInput":
            if name != pname:
                in_names.append(name)
        elif alloc.kind == "ExternalOutput":
            out_names.append(name)
            shape = tuple(alloc.tensor_shape)
            dtype = mybir.dt.np(alloc.dtype)
            out_avals.append(jax.core.ShapedArray(shape, dtype))
            zero_shapes.append((shape, dtype))
    all_in_names = in_names + out_names
    if pname is not None:
        all_in_names = all_in_names + [pname]

    def _body(*args):
        operands = list(args)
        if pname is not None:
            operands.append(partition_id_tensor())
        return tuple(_bass_exec_p.bind(
            *operands,
            out_avals=tuple(out_avals),
            in_names=tuple(all_in_names),
            out_names=tuple(out_names),
            lowering_input_output_aliases=(),
            sim_require_finite=True,
            sim_require_nnan=True,
            nc=nc,
        ))

    devices = jax.devices()[:NCORES]
    mesh = Mesh(np.asarray(devices), ("core",))
    shard = PartitionSpec("core")
    n_args = len(in_names) + len(out_names)
    sharded = jax.jit(
        shard_map(_body, mesh=mesh, in_specs=(shard,) * n_args,
                  out_specs=(shard,) * len(out_names), check_rep=False),
        keep_unused=True)
    zeros = [jax.device_put(np.zeros((NCORES * s[0], *s[1:]), d),
                            NamedSharding(mesh, shard))
             for (s, d) in zero_shapes]
    jax.block_until_ready(zeros)

    def run(in_maps):
        args = [
            jax.device_put(
                np.concatenate([np.asarray(m[n]) for m in in_maps], axis=0),
                NamedSharding(mesh, shard))
            for n in in_names
        ]
        outs = sharded(*args, *zeros)
        return [
            {n: np.asarray(outs[i]).reshape(NCORES, *out_avals[i].shape)[c]
             for i, n in enumerate(out_names)}
            for c in range(NCORES)
        ]

    return run


def kernel(x, wq, wk, wv, wo):
    x = np.asarray(x, dtype=np.float32)
    wq = np.asarray(wq, dtype=np.float32)
    wk = np.asarray(wk, dtype=np.float32)
    wv = np.asarray(wv, dtype=np.float32)
    wo = np.asarray(wo, dtype=np.float32)

    if "nc" not in _cache:
        _cache["nc"] = _build()
    nc = _cache["nc"]
    in_maps = make_in_maps(x, wq, wk, wv, wo)

    try:
        if "run" not in _cache:
            _cache["run"] = _make_runner(nc)
        results = _cache["run"](in_maps)
    except Exception:
        _cache.pop("run", None)
        results = run_bass_kernel_spmd(
            nc, in_maps, core_ids=list(range(NCORES))).results

    out = np.zeros((B, S, D), dtype=np.float64)
    for c, r in enumerate(results):
        b = c // 4
        out[b] += r["outp"].astype(np.float64)
    return out.astype(np.float32)
